# revision 7
# baseline (speedup 1.0000x reference)
"""ALIF (adaptive leaky integrate-and-fire) layer on 8 Trainium2 NeuronCores.

Pure data parallel: batch dim (8) -> one core each. Per core the [50, 64*64*32]
scan runs with all state resident in SBUF as [128, 1024] fp32 tiles:

    TH_t   = adp*TH_{t-1} + (beta*s_{t-1} + c)     c = 0.5*(1-adp)   (ACT+GPSIMD)
    v'_t   = d*v_{t-1} + x_t                                          (DVE STT)
    diff_t = v'_t - TH_t                                              (DVE)
    s_t    = diff_t > 0   -> uint8 (the output)                       (DVE)
    v_t    = diff_t where s_t else v'_t   (soft reset)                (DVE copy_predicated)

Spikes leave the device as uint8 and are cast to float32 on the host.
"""

import numpy as np

B, T, HH, WW, CC = 8, 50, 64, 64, 32
P, F = 128, 1024  # on-chip tile geometry; P*F == HH*WW*CC
XGRP = 4          # timesteps per x-load DMA / spike-store DMA

TRACE = False          # test harness may flip this to profile
LAST_EXEC_NS = None    # filled when TRACE is on

_CACHE = {}


def _apply_patches():
    """Environment workarounds for the walrus build in this container, which
    rejects any instruction carrying more than one semaphore wait."""
    import concourse.mybir as mybir
    import concourse.tile as tile

    if getattr(tile.TileContext, "_alif_patched", False):
        return

    _orig_add_instruction = tile.TileContext._add_instruction
    counter = [0]

    def _patched_add_instruction(self, inst):
        si = inst.sync_info
        if si is not None and len(si.on_wait) > 1:
            waits = list(si.on_wait)
            for w in waits[:-1]:
                counter[0] += 1
                ev = mybir.InstEventSemaphore(
                    name=f"wsplit-{counter[0]}",
                    ins=[],
                    outs=[],
                    sync_info=mybir.SyncInfo(on_wait=[w], on_update=[]),
                )
                ev.engine = inst.engine
                _orig_add_instruction(self, ev)
            inst.sync_info = mybir.SyncInfo(
                on_wait=[waits[-1]], on_update=list(si.on_update)
            )
        _orig_add_instruction(self, inst)

    def _patched_drain_and_barrier(self, tick_clock, wait_clock):
        gc = tick_clock.global_clock
        sems = self.sems.allocated()
        for proc_idx, sem in sorted(sems.items()):
            try:
                tick = gc[proc_idx]
            except Exception:
                continue
            if tick <= 0:
                continue
            name = getattr(sem, "name", "") or ""
            mult = 16 if "DMA" in name else 1
            self.nc.sync.wait_ge(sem, tick * mult)
        self.nc.sync.drain()
        self.nc.all_engine_barrier()
        popped = self.nc._tile_sem_poison_stack.pop()
        assert popped is self._sem_poison
        self.nc.clear_and_free_semaphores(list(self.sems.allocated().values()))
        self.nc.all_engine_barrier()

    tile.TileContext._add_instruction = _patched_add_instruction
    tile.TileContext._drain_and_barrier = _patched_drain_and_barrier
    tile.TileContext._alif_patched = True


def _build(d, adp, beta):
    import concourse.bass as bass
    import concourse.mybir as mybir
    import concourse.tile as tile

    ALU = mybir.AluOpType
    AF = mybir.ActivationFunctionType
    f32 = mybir.dt.float32
    u8 = mybir.dt.uint8

    c = 0.5 * (1.0 - adp)

    nc = bass.Bass()
    x_d = nc.dram_tensor("x", [P, T * F], f32, kind="ExternalInput")
    wadp_d = nc.dram_tensor("wadp", [P, P], f32, kind="ExternalInput")
    wi_d = nc.dram_tensor("wi", [P, P], f32, kind="ExternalInput")
    s_d = nc.dram_tensor("s8", [P, T * F], u8, kind="ExternalOutput")

    with tile.TileContext(nc) as tc:
        with (
            tc.tile_pool(name="xin", bufs=3) as xpool,
            tc.tile_pool(name="sout", bufs=2) as spool,
            tc.tile_pool(name="vst", bufs=2) as vpool,
            tc.tile_pool(name="thst", bufs=2) as thpool,
            tc.tile_pool(name="sbc", bufs=2) as sbcpool,
            tc.tile_pool(name="dif", bufs=2) as dpool,
            tc.tile_pool(name="ini", bufs=1) as ipool,
            tc.tile_pool(name="psth", bufs=2, space="PSUM") as pspool,
        ):
            v_init = ipool.tile([P, F], f32, name="v_init")
            th_init = ipool.tile([P, F], f32, name="th_init")
            wadp = ipool.tile([P, P], f32, name="wadp")
            wi = ipool.tile([P, P], f32, name="wi")
            nc.gpsimd.memset(v_init[:], 0.0)
            nc.gpsimd.memset(th_init[:], 0.5)
            nc.sync.dma_start(wadp[:], wadp_d[:])
            nc.sync.dma_start(wi[:], wi_d[:])

            v_prev = v_init
            th_prev = th_init  # SBUF copy of threshold (matmul rhs)
            sbc_prev = None
            x_tile = None
            s_tile = None

            for t in range(T):
                g = t % XGRP
                if g == 0:
                    n = min(XGRP, T - t)
                    x_tile = xpool.tile([P, XGRP * F], f32, name=f"xt{t}", tag="xt")
                    nc.sync.dma_start(
                        x_tile[:, : n * F], x_d[:, t * F : (t + n) * F]
                    )
                    s_tile = spool.tile([P, XGRP * F], u8, name=f"st{t}", tag="st")
                xa = x_tile[:, g * F : (g + 1) * F]

                # threshold track on the (otherwise idle) TensorEngine:
                # psum_th = adp*TH + 1.0*sbc  via diagonal weights. diff reads
                # the PSUM directly; an ACT evacuation provides the SBUF copy
                # needed as next step's matmul rhs. Off the v critical loop.
                if t == 0:
                    th_ps = None
                    th_sb = th_init
                else:
                    th_ps = pspool.tile([P, F], f32, name=f"thp{t}", tag="thp")
                    for ch in range(2):
                        sl = slice(ch * (F // 2), (ch + 1) * (F // 2))
                        nc.tensor.matmul(
                            th_ps[:, sl], wadp[:], th_prev[:, sl],
                            start=True, stop=False,
                        )
                    for ch in range(2):
                        sl = slice(ch * (F // 2), (ch + 1) * (F // 2))
                        nc.tensor.matmul(
                            th_ps[:, sl], wi[:], sbc_prev[:, sl],
                            start=False, stop=True,
                        )
                    th_sb = thpool.tile([P, F], f32, name=f"th{t}", tag="th")

                # v' = d*v + x_t                       (critical loop, DVE)
                v_new = vpool.tile([P, F], f32, name=f"v{t}", tag="v")
                nc.vector.scalar_tensor_tensor(
                    v_new[:], v_prev[:], d, xa, ALU.mult, ALU.add
                )
                # diff = v' - TH  (TH from PSUM when available)
                diff = dpool.tile([P, F], f32, name=f"diff{t}", tag="diff")
                th_src = th_ps if th_ps is not None else th_sb
                nc.vector.tensor_tensor(diff[:], v_new[:], th_src[:], ALU.subtract)
                # spikes (uint8): diff > 0
                s8v = s_tile[:, g * F : (g + 1) * F]
                nc.vector.tensor_scalar(s8v, diff[:], 0.0, None, ALU.is_gt)
                # sbc = beta*s + c for the next threshold update (ACT, urgent:
                # feeds next step's PE threshold matmul - emit before the
                # threshold evacuation so the in-order ACT queue runs it first)
                sbc_cur = sbcpool.tile([P, F], f32, name=f"sbc{t}", tag="sbc")
                nc.scalar.activation(
                    sbc_cur[:], s8v, AF.Copy, scale=beta, bias=c
                )
                if th_ps is not None:
                    nc.scalar.activation(th_sb[:], th_ps[:], AF.Copy)
                # soft reset: v'' = diff where spike else v'
                nc.vector.copy_predicated(v_new[:], s8v, diff[:])

                if g == XGRP - 1 or t == T - 1:
                    t0 = t - g
                    nc.sync.dma_start(
                        s_d[:, t0 * F : (t + 1) * F], s_tile[:, : (g + 1) * F]
                    )

                v_prev, th_prev, sbc_prev = v_new, th_sb, sbc_cur

    return nc


def kernel(x, hp_alif_d, hp_alif_adp, hp_alif_beta, hp_alpha):
    global LAST_EXEC_NS
    _apply_patches()
    from concourse.bass_utils import run_bass_kernel_spmd

    x = np.asarray(x, dtype=np.float32)
    assert x.shape == (B, T, HH, WW, CC), x.shape
    d = float(np.asarray(hp_alif_d))
    adp = float(np.asarray(hp_alif_adp))
    beta = float(np.asarray(hp_alif_beta))

    key = (d, adp, beta)
    if key not in _CACHE:
        _CACHE[key] = _build(d, adp, beta)
    nc = _CACHE[key]

    # per-core partition-major layout: [T, P, F] -> [P, T*F]
    in_maps = []
    for b in range(B):
        xh = (
            x[b]
            .reshape(T, P, F)
            .transpose(1, 0, 2)
            .reshape(P, T * F)
        )
        in_maps.append(
            {
                "x": np.ascontiguousarray(xh),
                "wadp": (np.eye(P) * adp).astype(np.float32),
                "wi": np.eye(P, dtype=np.float32),
            }
        )

    res = run_bass_kernel_spmd(
        nc, in_maps, core_ids=list(range(B)), trace=TRACE
    )
    LAST_EXEC_NS = res.exec_time_ns

    out = np.empty((B, T, HH, WW, CC), dtype=np.float32)
    for b in range(B):
        s8 = res.results[b]["s8"]  # [P, T*F] uint8
        out[b] = (
            s8.reshape(P, T, F)
            .transpose(1, 0, 2)
            .reshape(T, HH, WW, CC)
            .astype(np.float32)
        )
    return out


# revision 8
# speedup vs baseline: 1.0819x; 1.0819x over previous
"""ALIF (adaptive leaky integrate-and-fire) layer on 8 Trainium2 NeuronCores.

Pure data parallel: batch dim (8) -> one core each. Per core the [50, 64*64*32]
scan keeps all state in SBUF as [128, 1024] fp32 tiles.

Engine split per timestep (v-recurrence is a serial 4-op DVE loop; everything
else rides the otherwise-idle engines off the critical path):

  PE    psum_TH = c*I x ones + adp*I x TH_sb + beta*I x s01   (full threshold,
        exact fp32 matmuls with diagonal weights; constant injected via ones)
  ACT   TH_sb = copy(psum_TH)               (SBUF copy = next matmul rhs)
  DVE   v'  = d*v + x_t                     (scalar_tensor_tensor)
        dm  = v' - psum_TH                  (scalar_tensor_tensor, psum read)
        s01 = dm > 0                        (fp32 0/1; PE rhs + DMA source)
        v'' = dm where s01 else v'          (copy_predicated, int32-view mask)
  DMA   x in (4-step groups); spikes out via SWDGE fp32->uint8 casting DMA

Spikes reach DRAM as uint8 and are cast to float32 on the host.
"""

import numpy as np

B, T, HH, WW, CC = 8, 50, 64, 64, 32
P, F = 128, 1024  # on-chip tile geometry; P*F == HH*WW*CC
F2 = F // 2       # matmul moving-operand chunk (fp32 max 512)
XGRP = 4          # timesteps per x-load / spike-store DMA group

TRACE = False          # test harness may flip this to profile
LAST_EXEC_NS = None    # filled when TRACE is on

_CACHE = {}


def _apply_patches():
    """Environment workarounds for the walrus build in this container, which
    rejects any instruction carrying more than one semaphore wait."""
    import concourse.mybir as mybir
    import concourse.tile as tile

    if getattr(tile.TileContext, "_alif_patched", False):
        return

    _orig_add_instruction = tile.TileContext._add_instruction
    counter = [0]

    def _patched_add_instruction(self, inst):
        si = inst.sync_info
        if si is not None and len(si.on_wait) > 1:
            waits = list(si.on_wait)
            for w in waits[:-1]:
                counter[0] += 1
                ev = mybir.InstEventSemaphore(
                    name=f"wsplit-{counter[0]}",
                    ins=[],
                    outs=[],
                    sync_info=mybir.SyncInfo(on_wait=[w], on_update=[]),
                )
                ev.engine = inst.engine
                _orig_add_instruction(self, ev)
            inst.sync_info = mybir.SyncInfo(
                on_wait=[waits[-1]], on_update=list(si.on_update)
            )
        _orig_add_instruction(self, inst)

    def _patched_drain_and_barrier(self, tick_clock, wait_clock):
        gc = tick_clock.global_clock
        sems = self.sems.allocated()
        for proc_idx, sem in sorted(sems.items()):
            try:
                tick = gc[proc_idx]
            except Exception:
                continue
            if tick <= 0:
                continue
            name = getattr(sem, "name", "") or ""
            mult = 16 if "DMA" in name else 1
            self.nc.sync.wait_ge(sem, tick * mult)
        self.nc.sync.drain()
        self.nc.all_engine_barrier()
        popped = self.nc._tile_sem_poison_stack.pop()
        assert popped is self._sem_poison
        self.nc.clear_and_free_semaphores(list(self.sems.allocated().values()))
        self.nc.all_engine_barrier()

    tile.TileContext._add_instruction = _patched_add_instruction
    tile.TileContext._drain_and_barrier = _patched_drain_and_barrier
    tile.TileContext._alif_patched = True


def _build(d, adp, beta):
    import concourse.bass as bass
    import concourse.mybir as mybir
    import concourse.tile as tile

    ALU = mybir.AluOpType
    AF = mybir.ActivationFunctionType
    f32 = mybir.dt.float32
    i32 = mybir.dt.int32
    u8 = mybir.dt.uint8

    nc = bass.Bass()
    x_d = nc.dram_tensor("x", [P, T * F], f32, kind="ExternalInput")
    wadp_d = nc.dram_tensor("wadp", [P, P], f32, kind="ExternalInput")
    wbeta_d = nc.dram_tensor("wbeta", [P, P], f32, kind="ExternalInput")
    wc_d = nc.dram_tensor("wc", [P, P], f32, kind="ExternalInput")
    s_d = nc.dram_tensor("s8", [P, T * F], u8, kind="ExternalOutput")

    with tile.TileContext(nc) as tc:
        with (
            tc.tile_pool(name="xin", bufs=3) as xpool,
            tc.tile_pool(name="sout", bufs=2) as spool,
            tc.tile_pool(name="vst", bufs=2) as vpool,
            tc.tile_pool(name="thst", bufs=2) as thpool,
            tc.tile_pool(name="dif", bufs=2) as dpool,
            tc.tile_pool(name="ini", bufs=1) as ipool,
            tc.tile_pool(name="psth", bufs=3, space="PSUM") as pspool,
        ):
            v_init = ipool.tile([P, F], f32, name="v_init")
            th_init = ipool.tile([P, F], f32, name="th_init")
            ones = ipool.tile([P, F2], f32, name="ones")
            wadp = ipool.tile([P, P], f32, name="wadp")
            wbeta = ipool.tile([P, P], f32, name="wbeta")
            wc = ipool.tile([P, P], f32, name="wc")
            nc.gpsimd.memset(v_init[:], 0.0)
            nc.gpsimd.memset(th_init[:], 0.5)
            nc.gpsimd.memset(ones[:], 1.0)
            nc.sync.dma_start(wadp[:], wadp_d[:])
            nc.sync.dma_start(wbeta[:], wbeta_d[:])
            nc.sync.dma_start(wc[:], wc_d[:])

            v_prev = v_init
            th_prev = th_init   # threshold SBUF copy (matmul rhs)
            s_prev = None       # previous spikes, fp32 0/1 (matmul rhs)
            x_tile = None
            s_tile = None

            for t in range(T):
                g = t % XGRP
                if g == 0:
                    n = min(XGRP, T - t)
                    x_tile = xpool.tile([P, XGRP * F], f32, name=f"xt{t}", tag="xt")
                    nc.sync.dma_start(
                        x_tile[:, : n * F], x_d[:, t * F : (t + n) * F]
                    )
                    s_tile = spool.tile([P, XGRP * F], f32, name=f"st{t}", tag="st")
                xa = x_tile[:, g * F : (g + 1) * F]

                # full threshold on the TensorEngine:
                #   psum_TH = c*I x ones + adp*I x TH_prev + beta*I x s_prev
                th_ps = None
                if t > 0:
                    th_ps = pspool.tile([P, F], f32, name=f"thp{t}", tag="thp")
                    for ch in range(2):
                        sl = slice(ch * F2, (ch + 1) * F2)
                        nc.tensor.matmul(
                            th_ps[:, sl], wc[:], ones[:], start=True, stop=False
                        )
                    for ch in range(2):
                        sl = slice(ch * F2, (ch + 1) * F2)
                        nc.tensor.matmul(
                            th_ps[:, sl], wadp[:], th_prev[:, sl],
                            start=False, stop=False,
                        )
                    for ch in range(2):
                        sl = slice(ch * F2, (ch + 1) * F2)
                        nc.tensor.matmul(
                            th_ps[:, sl], wbeta[:], s_prev[:, sl],
                            start=False, stop=True,
                        )

                # ---- the serial v-loop, all on DVE ----
                v_new = vpool.tile([P, F], f32, name=f"v{t}", tag="v")
                nc.vector.scalar_tensor_tensor(
                    v_new[:], v_prev[:], d, xa, ALU.mult, ALU.add
                )
                dm = dpool.tile([P, F], f32, name=f"dm{t}", tag="dm")
                if th_ps is not None:
                    nc.vector.scalar_tensor_tensor(
                        dm[:], th_ps[:], -1.0, v_new[:], ALU.mult, ALU.add
                    )
                else:
                    nc.vector.tensor_tensor(
                        dm[:], v_new[:], th_init[:], ALU.subtract
                    )
                s01 = s_tile[:, g * F : (g + 1) * F]
                nc.vector.tensor_scalar(s01, dm[:], 0.0, None, ALU.is_gt)
                nc.vector.copy_predicated(v_new[:], s01.bitcast(i32), dm[:])
                # ---------------------------------------

                # threshold SBUF copy for the next matmul (off-loop, ACT)
                if th_ps is not None:
                    th_sb = thpool.tile([P, F], f32, name=f"th{t}", tag="th")
                    nc.scalar.activation(th_sb[:], th_ps[:], AF.Copy)
                else:
                    th_sb = th_init

                if g == XGRP - 1 or t == T - 1:
                    t0 = t - g
                    # SWDGE casting DMA: fp32 {0,1} -> uint8
                    nc.gpsimd.dma_start(
                        s_d[:, t0 * F : (t + 1) * F], s_tile[:, : (g + 1) * F]
                    )

                v_prev, th_prev, s_prev = v_new, th_sb, s01

    return nc


def kernel(x, hp_alif_d, hp_alif_adp, hp_alif_beta, hp_alpha):
    global LAST_EXEC_NS
    _apply_patches()
    from concourse.bass_utils import run_bass_kernel_spmd

    x = np.asarray(x, dtype=np.float32)
    assert x.shape == (B, T, HH, WW, CC), x.shape
    d = float(np.asarray(hp_alif_d))
    adp = float(np.asarray(hp_alif_adp))
    beta = float(np.asarray(hp_alif_beta))
    c = 0.5 * (1.0 - adp)

    key = (d, adp, beta)
    if key not in _CACHE:
        _CACHE[key] = _build(d, adp, beta)
    nc = _CACHE[key]

    # per-core partition-major layout: [T, P, F] -> [P, T*F]
    in_maps = []
    for b in range(B):
        xh = (
            x[b]
            .reshape(T, P, F)
            .transpose(1, 0, 2)
            .reshape(P, T * F)
        )
        in_maps.append(
            {
                "x": np.ascontiguousarray(xh),
                "wadp": (np.eye(P) * adp).astype(np.float32),
                "wbeta": (np.eye(P) * beta).astype(np.float32),
                "wc": (np.eye(P) * c).astype(np.float32),
            }
        )

    res = run_bass_kernel_spmd(
        nc, in_maps, core_ids=list(range(B)), trace=TRACE
    )
    LAST_EXEC_NS = res.exec_time_ns

    out = np.empty((B, T, HH, WW, CC), dtype=np.float32)
    for b in range(B):
        s8 = res.results[b]["s8"]  # [P, T*F] uint8
        out[b] = (
            s8.reshape(P, T, F)
            .transpose(1, 0, 2)
            .reshape(T, HH, WW, CC)
            .astype(np.float32)
        )
    return out


# revision 14
# speedup vs baseline: 1.1967x; 1.1061x over previous
"""ALIF (adaptive leaky integrate-and-fire) layer on 8 Trainium2 NeuronCores.

Pure data parallel: batch dim (8) -> one core each. Per core the [50, 64*64*32]
scan keeps all state in SBUF as [128, 1024] fp32 tiles.

Engine split per timestep (v-recurrence is a serial 4-op DVE loop; everything
else rides the otherwise-idle engines off the critical path):

  PE    psum_TH = c*I x ones + adp*I x TH_sb + beta*I x s01   (full threshold,
        exact fp32 matmuls with diagonal weights; constant injected via ones)
  ACT   TH_sb = copy(psum_TH)               (SBUF copy = next matmul rhs)
  DVE   v'  = d*v + x_t                     (scalar_tensor_tensor)
        dm  = v' - psum_TH                  (scalar_tensor_tensor, psum read)
        s01 = dm > 0                        (fp32 0/1; PE rhs + DMA source)
        v'' = dm where s01 else v'          (copy_predicated, int32-view mask)
  DMA   x in (4-step groups); spikes out via SWDGE fp32->uint8 casting DMA

Spikes reach DRAM as uint8 and are cast to float32 on the host.
"""

import numpy as np

B, T, HH, WW, CC = 8, 50, 64, 64, 32
P, F = 128, 1024  # on-chip tile geometry; P*F == HH*WW*CC
F2 = F // 2       # matmul moving-operand chunk (fp32 max 512)
XGRP = 4          # timesteps per x-load / spike-store DMA group

TRACE = False          # test harness may flip this to profile
LAST_EXEC_NS = None    # filled when TRACE is on

_CACHE = {}


def _apply_patches():
    """Environment workarounds for the walrus build in this container, which
    rejects any instruction carrying more than one semaphore wait."""
    import concourse.mybir as mybir
    import concourse.tile as tile

    if getattr(tile.TileContext, "_alif_patched", False):
        return

    _orig_add_instruction = tile.TileContext._add_instruction
    counter = [0]

    def _patched_add_instruction(self, inst):
        si = inst.sync_info
        if si is not None and len(si.on_wait) > 1:
            waits = list(si.on_wait)
            for w in waits[:-1]:
                counter[0] += 1
                ev = mybir.InstEventSemaphore(
                    name=f"wsplit-{counter[0]}",
                    ins=[],
                    outs=[],
                    sync_info=mybir.SyncInfo(on_wait=[w], on_update=[]),
                )
                ev.engine = inst.engine
                _orig_add_instruction(self, ev)
            inst.sync_info = mybir.SyncInfo(
                on_wait=[waits[-1]], on_update=list(si.on_update)
            )
        _orig_add_instruction(self, inst)

    def _patched_drain_and_barrier(self, tick_clock, wait_clock):
        gc = tick_clock.global_clock
        sems = self.sems.allocated()
        for proc_idx, sem in sorted(sems.items()):
            try:
                tick = gc[proc_idx]
            except Exception:
                continue
            if tick <= 0:
                continue
            name = getattr(sem, "name", "") or ""
            mult = 16 if "DMA" in name else 1
            self.nc.sync.wait_ge(sem, tick * mult)
        self.nc.sync.drain()
        self.nc.all_engine_barrier()
        popped = self.nc._tile_sem_poison_stack.pop()
        assert popped is self._sem_poison
        self.nc.clear_and_free_semaphores(list(self.sems.allocated().values()))
        self.nc.all_engine_barrier()

    tile.TileContext._add_instruction = _patched_add_instruction
    tile.TileContext._drain_and_barrier = _patched_drain_and_barrier
    tile.TileContext._alif_patched = True


def _build(d, adp, beta):
    import concourse.bass as bass
    import concourse.mybir as mybir
    import concourse.tile as tile

    ALU = mybir.AluOpType
    AF = mybir.ActivationFunctionType
    f32 = mybir.dt.float32
    i32 = mybir.dt.int32
    u8 = mybir.dt.uint8

    adp = float(adp)
    c = 0.5 * (1.0 - adp)
    gamma = c / adp  # evac bias: adp*(psum + gamma) == adp*psum + c

    nc = bass.Bass()
    x_d = nc.dram_tensor("x", [P, T * F], f32, kind="ExternalInput")
    wadp_d = nc.dram_tensor("wadp", [P, P], f32, kind="ExternalInput")
    wbeta_d = nc.dram_tensor("wbeta", [P, P], f32, kind="ExternalInput")
    s_d = nc.dram_tensor("s8", [P, T * F], u8, kind="ExternalOutput")

    with tile.TileContext(nc) as tc:
        with (
            tc.tile_pool(name="xin", bufs=3) as xpool,
            tc.tile_pool(name="sout", bufs=2) as spool,
            tc.tile_pool(name="vst", bufs=2) as vpool,
            tc.tile_pool(name="thst", bufs=2) as thpool,
            tc.tile_pool(name="dif", bufs=2) as dpool,
            tc.tile_pool(name="ini", bufs=1) as ipool,
            tc.tile_pool(name="psth", bufs=3, space="PSUM") as pspool,
        ):
            v_init = ipool.tile([P, F], f32, name="v_init")
            th_init = ipool.tile([P, F], f32, name="th_init")
            thsbc_init = ipool.tile([P, F], f32, name="thsbc_init")
            wadp = ipool.tile([P, P], f32, name="wadp")
            wbeta = ipool.tile([P, P], f32, name="wbeta")
            nc.gpsimd.memset(v_init[:], 0.0)
            nc.gpsimd.memset(th_init[:], 0.5)
            nc.gpsimd.memset(thsbc_init[:], 0.5 + gamma)
            nc.sync.dma_start(wadp[:], wadp_d[:])
            nc.sync.dma_start(wbeta[:], wbeta_d[:])

            v_prev = v_init
            th_prev = thsbc_init  # biased threshold SBUF copy (matmul rhs)
            s_prev = None         # previous spikes, fp32 0/1 (matmul rhs)
            x_tile = None
            s_tile = None

            for t in range(T):
                g = t % XGRP
                if g == 0:
                    n = min(XGRP, T - t)
                    x_tile = xpool.tile([P, XGRP * F], f32, name=f"xt{t}", tag="xt")
                    nc.sync.dma_start(
                        x_tile[:, : n * F], x_d[:, t * F : (t + n) * F]
                    )
                    s_tile = spool.tile([P, XGRP * F], f32, name=f"st{t}", tag="st")
                xa = x_tile[:, g * F : (g + 1) * F]

                # full threshold on the TensorEngine:
                #   psum_TH = adp*I x (TH_prev + c/adp) + beta*I x s_prev
                #           = adp*TH + beta*s + c  (the +c rides the evac bias)
                th_ps = None
                if t > 0:
                    th_ps = pspool.tile([P, F], f32, name=f"thp{t}", tag="thp")
                    for ch in range(2):
                        sl = slice(ch * F2, (ch + 1) * F2)
                        nc.tensor.matmul(
                            th_ps[:, sl], wadp[:], th_prev[:, sl],
                            start=True, stop=False,
                        )
                    for ch in range(2):
                        sl = slice(ch * F2, (ch + 1) * F2)
                        nc.tensor.matmul(
                            th_ps[:, sl], wbeta[:], s_prev[:, sl],
                            start=False, stop=True,
                        )

                # ---- the serial v-loop, all on DVE ----
                v_new = vpool.tile([P, F], f32, name=f"v{t}", tag="v")
                nc.vector.scalar_tensor_tensor(
                    v_new[:], v_prev[:], d, xa, ALU.mult, ALU.add
                )
                dm = dpool.tile([P, F], f32, name=f"dm{t}", tag="dm")
                if th_ps is not None:
                    nc.vector.scalar_tensor_tensor(
                        dm[:], th_ps[:], -1.0, v_new[:], ALU.mult, ALU.add
                    )
                else:
                    nc.vector.tensor_tensor(
                        dm[:], v_new[:], th_init[:], ALU.subtract
                    )
                s01 = s_tile[:, g * F : (g + 1) * F]
                nc.vector.tensor_scalar(s01, dm[:], 0.0, None, ALU.is_gt)
                nc.vector.copy_predicated(v_new[:], s01.bitcast(i32), dm[:])
                # ---------------------------------------

                # biased threshold SBUF copy for the next matmul (off-loop, ACT)
                if th_ps is not None:
                    th_sb = thpool.tile([P, F], f32, name=f"th{t}", tag="th")
                    nc.scalar.activation(th_sb[:], th_ps[:], AF.Copy, bias=gamma)
                else:
                    th_sb = thsbc_init

                if g == XGRP - 1 or t == T - 1:
                    t0 = t - g
                    # SWDGE casting DMA: fp32 {0,1} -> uint8
                    nc.gpsimd.dma_start(
                        s_d[:, t0 * F : (t + 1) * F], s_tile[:, : (g + 1) * F]
                    )

                v_prev, th_prev, s_prev = v_new, th_sb, s01

    return nc


def kernel(x, hp_alif_d, hp_alif_adp, hp_alif_beta, hp_alpha):
    global LAST_EXEC_NS
    _apply_patches()
    from concourse.bass_utils import run_bass_kernel_spmd

    x = np.asarray(x, dtype=np.float32)
    assert x.shape == (B, T, HH, WW, CC), x.shape
    d = float(np.asarray(hp_alif_d))
    adp = float(np.asarray(hp_alif_adp))
    beta = float(np.asarray(hp_alif_beta))
    c = 0.5 * (1.0 - adp)

    key = (d, adp, beta)
    if key not in _CACHE:
        _CACHE[key] = _build(d, adp, beta)
    nc = _CACHE[key]

    # per-core partition-major layout: [T, P, F] -> [P, T*F]
    in_maps = []
    for b in range(B):
        xh = (
            x[b]
            .reshape(T, P, F)
            .transpose(1, 0, 2)
            .reshape(P, T * F)
        )
        in_maps.append(
            {
                "x": np.ascontiguousarray(xh),
                "wadp": (np.eye(P) * adp).astype(np.float32),
                "wbeta": (np.eye(P) * beta).astype(np.float32),
            }
        )

    res = run_bass_kernel_spmd(
        nc, in_maps, core_ids=list(range(B)), trace=TRACE
    )
    LAST_EXEC_NS = res.exec_time_ns

    out = np.empty((B, T, HH, WW, CC), dtype=np.float32)
    for b in range(B):
        s8 = res.results[b]["s8"]  # [P, T*F] uint8
        out[b] = (
            s8.reshape(P, T, F)
            .transpose(1, 0, 2)
            .reshape(T, HH, WW, CC)
            .astype(np.float32)
        )
    return out


# revision 20
# speedup vs baseline: 1.6700x; 1.3954x over previous
"""ALIF (adaptive leaky integrate-and-fire) layer on 8 Trainium2 NeuronCores.

Pure data parallel: batch dim (8) -> one core each. Per core the [50, 64*64*32]
scan keeps all state in SBUF as [128, 1024] fp32 tiles.

Engine split per timestep (v-recurrence is a serial 4-op DVE loop; everything
else rides the otherwise-idle engines off the critical path):

  PE    psum_TH = c*I x ones + adp*I x TH_sb + beta*I x s01   (full threshold,
        exact fp32 matmuls with diagonal weights; constant injected via ones)
  ACT   TH_sb = copy(psum_TH)               (SBUF copy = next matmul rhs)
  DVE   v'  = d*v + x_t                     (scalar_tensor_tensor)
        dm  = v' - psum_TH                  (scalar_tensor_tensor, psum read)
        s01 = dm > 0                        (fp32 0/1; PE rhs + DMA source)
        v'' = dm where s01 else v'          (copy_predicated, int32-view mask)
  DMA   x in (4-step groups); spikes out via SWDGE fp32->uint8 casting DMA

Spikes reach DRAM as uint8 and are cast to float32 on the host.
"""

import numpy as np

B, T, HH, WW, CC = 8, 50, 64, 64, 32
P, F = 128, 1024  # on-chip tile geometry; P*F == HH*WW*CC
F2 = F // 2       # matmul moving-operand chunk (fp32 max 512)
XGRP = 4          # timesteps per x-load / spike-store DMA group

TRACE = False          # test harness may flip this to profile
LAST_EXEC_NS = None    # filled when TRACE is on

_CACHE = {}


def _apply_patches():
    """Environment workarounds for the walrus build in this container, which
    rejects any instruction carrying more than one semaphore wait."""
    import concourse.mybir as mybir
    import concourse.tile as tile

    if getattr(tile.TileContext, "_alif_patched", False):
        return

    _orig_add_instruction = tile.TileContext._add_instruction
    counter = [0]

    def _patched_add_instruction(self, inst):
        si = inst.sync_info
        if si is not None and len(si.on_wait) > 1:
            waits = list(si.on_wait)
            for w in waits[:-1]:
                counter[0] += 1
                ev = mybir.InstEventSemaphore(
                    name=f"wsplit-{counter[0]}",
                    ins=[],
                    outs=[],
                    sync_info=mybir.SyncInfo(on_wait=[w], on_update=[]),
                )
                ev.engine = inst.engine
                _orig_add_instruction(self, ev)
            inst.sync_info = mybir.SyncInfo(
                on_wait=[waits[-1]], on_update=list(si.on_update)
            )
        _orig_add_instruction(self, inst)

    def _patched_drain_and_barrier(self, tick_clock, wait_clock):
        gc = tick_clock.global_clock
        sems = self.sems.allocated()
        for proc_idx, sem in sorted(sems.items()):
            try:
                tick = gc[proc_idx]
            except Exception:
                continue
            if tick <= 0:
                continue
            name = getattr(sem, "name", "") or ""
            mult = 16 if "DMA" in name else 1
            self.nc.sync.wait_ge(sem, tick * mult)
        self.nc.sync.drain()
        self.nc.all_engine_barrier()
        popped = self.nc._tile_sem_poison_stack.pop()
        assert popped is self._sem_poison
        self.nc.clear_and_free_semaphores(list(self.sems.allocated().values()))
        self.nc.all_engine_barrier()

    tile.TileContext._add_instruction = _patched_add_instruction
    tile.TileContext._drain_and_barrier = _patched_drain_and_barrier
    tile.TileContext._alif_patched = True


def _build(d, adp, beta):
    import concourse.bass as bass
    import concourse.mybir as mybir
    import concourse.tile as tile

    ALU = mybir.AluOpType
    AF = mybir.ActivationFunctionType
    f32 = mybir.dt.float32
    i32 = mybir.dt.int32
    u8 = mybir.dt.uint8

    adp = float(adp)
    c = 0.5 * (1.0 - adp)
    gamma = c / adp  # evac bias: adp*(psum + gamma) == adp*psum + c

    nc = bass.Bass()
    x_d = nc.dram_tensor("x", [P, T * F], f32, kind="ExternalInput")
    wadp_d = nc.dram_tensor("wadp", [P, P], f32, kind="ExternalInput")
    wbeta_d = nc.dram_tensor("wbeta", [P, P], f32, kind="ExternalInput")
    s_d = nc.dram_tensor("s8", [P, T * F], u8, kind="ExternalOutput")

    with tile.TileContext(nc) as tc:
        with (
            tc.tile_pool(name="xin", bufs=3) as xpool,
            tc.tile_pool(name="sout", bufs=2) as spool,
            tc.tile_pool(name="vst", bufs=2) as vpool,
            tc.tile_pool(name="thst", bufs=2) as thpool,
            tc.tile_pool(name="dif", bufs=2) as dpool,
            tc.tile_pool(name="ini", bufs=1) as ipool,
            tc.tile_pool(name="psth", bufs=3, space="PSUM") as pspool,
        ):
            v_init = ipool.tile([P, F], f32, name="v_init")
            th_init = ipool.tile([P, F], f32, name="th_init")
            thsbc_init = ipool.tile([P, F], f32, name="thsbc_init")
            wadp = ipool.tile([P, P], f32, name="wadp")
            wbeta = ipool.tile([P, P], f32, name="wbeta")
            nc.gpsimd.memset(v_init[:], 0.0)
            nc.gpsimd.memset(th_init[:], 0.5)
            nc.gpsimd.memset(thsbc_init[:], 0.5 + gamma)
            nc.sync.dma_start(wadp[:], wadp_d[:])
            nc.sync.dma_start(wbeta[:], wbeta_d[:])

            v_prev = v_init
            th_prev = thsbc_init  # biased threshold SBUF copy (matmul rhs)
            s_prev = None         # previous spikes, fp32 0/1 (matmul rhs)
            x_tile = None
            s_tile = None

            for t in range(T):
                g = t % XGRP
                if g == 0:
                    n = min(XGRP, T - t)
                    x_tile = xpool.tile([P, XGRP * F], f32, name=f"xt{t}", tag="xt")
                    nc.sync.dma_start(
                        x_tile[:, : n * F], x_d[:, t * F : (t + n) * F]
                    )
                    s_tile = spool.tile([P, XGRP * F], f32, name=f"st{t}", tag="st")
                xa = x_tile[:, g * F : (g + 1) * F]

                # full threshold on the TensorEngine:
                #   psum_TH = adp*I x (TH_prev + c/adp) + beta*I x s_prev
                #           = adp*TH + beta*s + c  (the +c rides the evac bias)
                th_ps = None
                if t > 0:
                    # one PSUM tile per 512-chunk: keeps PE writes and DVE
                    # reads on disjoint tiles/banks (same-bank PE-W + DVE-R
                    # is a hardware-fatal race)
                    th_ps = [
                        pspool.tile([P, F2], f32, name=f"thp{t}_{ch}", tag=f"thp{ch}")
                        for ch in range(2)
                    ]
                    for ch in range(2):
                        sl = slice(ch * F2, (ch + 1) * F2)
                        nc.tensor.matmul(
                            th_ps[ch][:], wadp[:], th_prev[:, sl],
                            start=True, stop=False,
                        )
                    # beta matmuls last: they are the ones gated on the
                    # previous step's spikes
                    for ch in range(2):
                        sl = slice(ch * F2, (ch + 1) * F2)
                        nc.tensor.matmul(
                            th_ps[ch][:], wbeta[:], s_prev[:, sl],
                            start=False, stop=True,
                        )

                # ---- the serial v-loop, all on DVE ----
                v_new = vpool.tile([P, F], f32, name=f"v{t}", tag="v")
                nc.vector.scalar_tensor_tensor(
                    v_new[:], v_prev[:], d, xa, ALU.mult, ALU.add
                )
                dm = dpool.tile([P, F], f32, name=f"dm{t}", tag="dm")
                if th_ps is not None:
                    # chunk-split so dm[c0] proceeds as soon as the c0 psum
                    # accumulation group is done (finer cross-engine pipeline)
                    for ch in range(2):
                        sl = slice(ch * F2, (ch + 1) * F2)
                        nc.vector.scalar_tensor_tensor(
                            dm[:, sl], th_ps[ch][:], -1.0, v_new[:, sl],
                            ALU.mult, ALU.add,
                        )
                else:
                    nc.vector.tensor_tensor(
                        dm[:], v_new[:], th_init[:], ALU.subtract
                    )
                s01 = s_tile[:, g * F : (g + 1) * F]
                nc.vector.tensor_scalar(s01, dm[:], 0.0, None, ALU.is_gt)
                nc.vector.copy_predicated(v_new[:], s01.bitcast(i32), dm[:])
                # ---------------------------------------

                # biased threshold SBUF copy for the next matmul (off-loop, ACT)
                if th_ps is not None:
                    th_sb = thpool.tile([P, F], f32, name=f"th{t}", tag="th")
                    for ch in range(2):
                        sl = slice(ch * F2, (ch + 1) * F2)
                        nc.scalar.activation(
                            th_sb[:, sl], th_ps[ch][:], AF.Copy, bias=gamma
                        )
                else:
                    th_sb = thsbc_init

                if g == XGRP - 1 or t == T - 1:
                    t0 = t - g
                    # SWDGE casting DMA: fp32 {0,1} -> uint8
                    nc.gpsimd.dma_start(
                        s_d[:, t0 * F : (t + 1) * F], s_tile[:, : (g + 1) * F]
                    )

                v_prev, th_prev, s_prev = v_new, th_sb, s01

    return nc


def kernel(x, hp_alif_d, hp_alif_adp, hp_alif_beta, hp_alpha):
    global LAST_EXEC_NS
    _apply_patches()
    from concourse.bass_utils import run_bass_kernel_spmd

    x = np.asarray(x, dtype=np.float32)
    assert x.shape == (B, T, HH, WW, CC), x.shape
    d = float(np.asarray(hp_alif_d))
    adp = float(np.asarray(hp_alif_adp))
    beta = float(np.asarray(hp_alif_beta))
    c = 0.5 * (1.0 - adp)

    key = (d, adp, beta)
    if key not in _CACHE:
        _CACHE[key] = _build(d, adp, beta)
    nc = _CACHE[key]

    # per-core partition-major layout: [T, P, F] -> [P, T*F]
    in_maps = []
    for b in range(B):
        xh = (
            x[b]
            .reshape(T, P, F)
            .transpose(1, 0, 2)
            .reshape(P, T * F)
        )
        in_maps.append(
            {
                "x": np.ascontiguousarray(xh),
                "wadp": (np.eye(P) * adp).astype(np.float32),
                "wbeta": (np.eye(P) * beta).astype(np.float32),
            }
        )

    res = run_bass_kernel_spmd(
        nc, in_maps, core_ids=list(range(B)), trace=TRACE
    )
    LAST_EXEC_NS = res.exec_time_ns

    out = np.empty((B, T, HH, WW, CC), dtype=np.float32)
    for b in range(B):
        s8 = res.results[b]["s8"]  # [P, T*F] uint8
        out[b] = (
            s8.reshape(P, T, F)
            .transpose(1, 0, 2)
            .reshape(T, HH, WW, CC)
            .astype(np.float32)
        )
    return out
